# revision 33
# baseline (speedup 1.0000x reference)
"""Trainium2 Bass kernel for nn_Attention_41420664603305.

Multi-head self-attention block (16 heads, d_model=1024, head_dim=64, no
score scaling) with residual:  y = x + MHA(x).

Sharding over 8 NeuronCores: core c handles batch b = c//2 and heads
h0 = (c%2)*8 .. h0+8  (tensor-parallel column split of Wq/Wk/Wv).  Each core
computes y[b, :, (c%2)*512 : (c%2)*512+512] transposed; host re-transposes
and concatenates.

On-chip formulation (per core; T=2048 tokens, M=512 out cols, 8 heads):
  - Host pre-transposes: xT [1024, 2048], w{q,k,v}T [1024, 512],
    xresbT [512, 2048] (= (x residual + bv)^T; bv folds into the residual
    because softmax rows sum to exactly 1).
  - qT, kT = wT.T @ xT  ([m, t] layout, bias added per-partition on DVE).
  - v = xT.T @ wvT  ([t, m] layout), stored per-head as 65-wide groups with a
    trailing ones column, so attn@V also produces the softmax denominator.
  - Attention runs over head PAIRS (2j, 2j+1): the two heads' score matmuls
    have 64-deep contraction at partition offsets 0 / 64, so they lower to
    the two 64x128 PE row-tiles and execute concurrently.
    Per (pair, q-half, k-block): scoresT[k, q] (f32r) -> exp on ScalarE
    (no max subtraction; |scores| < ~50 fits fp32/exp fine) -> attnV
    accumulates attT[65, q] over k-blocks (row 64 = sums).
  - Normalize without transposes: reciprocal in place on attT row 64, then
    a selector matmul (sel.T @ att, sel = e_64 outer ones) broadcasts the
    recip row across partitions in PSUM; one DVE multiply writes the
    normalized head output straight into yT [m, t].
  - yT += xresbT, DMA out (y delivered transposed [M, T]).

All big matmuls run as float32r (full-rate fp32 mode on the PE).
"""

import sys

if "/opt/trn_rl_repo" not in sys.path:
    sys.path.insert(0, "/opt/trn_rl_repo")

import numpy as np

B, N, D, HEADS = 4, 2048, 1024, 16
HD = 64
N_CORES = 8

FULL_CFG = dict(T=N, DIN=D, M=D // 2, NH=HEADS // 2)


def build_program(T, DIN, M, NH, n_devices=N_CORES, enable_asserts=False):
    """Emit the Tile program; same SPMD program runs on every core."""
    from contextlib import ExitStack

    import concourse.bass as bass  # noqa: F401
    import concourse.tile as tile
    from concourse import bacc, mybir
    from concourse.masks import make_identity  # noqa: F401

    P = 128
    HG = HD + 1                    # per-head group width in v (64 vals + one)
    f32 = mybir.dt.float32
    f32r = mybir.dt.float32r
    bf16 = mybir.dt.bfloat16
    DC = DIN // P                  # contraction chunks
    MB = M // P                    # m-blocks (2 heads each)
    TB = T // P                    # token blocks
    TC = min(512, T)               # moving chunk for projections
    NTC = T // TC
    QH = T // 2                    # q processed in halves
    QC = min(512, QH)              # scores/attnV moving chunk
    NQC = QH // QC

    nc = bacc.Bacc("TRN2", target_bir_lowering=False, debug=False,
                   enable_asserts=enable_asserts, num_devices=n_devices)

    xT = nc.dram_tensor("xT", [DIN, T], f32r, kind="ExternalInput").ap()
    wqT = nc.dram_tensor("wqT", [DIN, M], f32r, kind="ExternalInput").ap()
    wkT = nc.dram_tensor("wkT", [DIN, M], f32r, kind="ExternalInput").ap()
    wvT = nc.dram_tensor("wvT", [DIN, M], f32r, kind="ExternalInput").ap()
    bqT = nc.dram_tensor("bqT", [P, MB], f32, kind="ExternalInput").ap()
    bkT = nc.dram_tensor("bkT", [P, MB], f32, kind="ExternalInput").ap()
    xresbT = nc.dram_tensor("xresbT", [M, T], f32, kind="ExternalInput").ap()
    y = nc.dram_tensor("y", [M, T], f32, kind="ExternalOutput").ap()

    with tile.TileContext(nc) as tc, ExitStack() as ctx:
        singles = ctx.enter_context(tc.tile_pool(name="singles", bufs=1))
        xp = ctx.enter_context(tc.tile_pool(name="xp", bufs=min(10, 2 * DC)))
        big = ctx.enter_context(tc.tile_pool(name="big", bufs=6))
        attp = ctx.enter_context(tc.tile_pool(name="attp", bufs=2))
        qk = ctx.enter_context(tc.tile_pool(name="qk", bufs=MB))
        vp = ctx.enter_context(tc.tile_pool(name="vp", bufs=TB))
        wp = ctx.enter_context(tc.tile_pool(name="wp", bufs=min(16, 3 * DC)))
        yp = ctx.enter_context(tc.tile_pool(name="yp", bufs=MB))

        # constants: selector (row HD = ones, else 0) and per-partition biases
        sel = singles.tile([P, HG], f32r, tag="sel")
        nc.vector.memset(sel[:].bitcast(f32), 0.0)
        nc.vector.memset(sel[HD:HG, :].bitcast(f32), 1.0)
        bq_t = singles.tile([P, MB], f32, tag="bq")
        nc.sync.dma_start(bq_t[:], bqT[:, :])
        bk_t = singles.tile([P, MB], f32, tag="bk")
        nc.sync.dma_start(bk_t[:], bkT[:, :])

        def load_x(dc, tci, tag):
            t = xp.tile([P, TC], f32r, tag="x", name=f"x{tag}_{dc}_{tci}")
            eng = nc.sync if dc % 2 == 0 else nc.gpsimd
            eng.dma_start(t[:], xT[dc * P:(dc + 1) * P,
                                   tci * TC:(tci + 1) * TC])
            return t

        def load_w(w_ap, dc):
            t = wp.tile([P, M], f32r, tag="w")
            eng = nc.gpsimd if dc % 2 == 0 else nc.sync
            eng.dma_start(t[:], w_ap[dc * P:(dc + 1) * P, :])
            return t

        qT_t = [qk.tile([P, T], f32r, tag="qT", name=f"qT{i}") for i in range(MB)]
        kT_t = [qk.tile([P, T], f32r, tag="kT", name=f"kT{i}") for i in range(MB)]
        v_t = [vp.tile([P, NH * HG], bf16, tag="v", name=f"v{i}") for i in range(TB)]
        yT_t = [yp.tile([P, T], f32, tag="y", name=f"yT{i}") for i in range(MB)]

        # ---- projections (x streamed per pass in TC-column chunks) ----
        TBC = TC // P  # token blocks per x chunk column group
        with tc.tile_pool(name="pps", bufs=6, space="PSUM") as pps:
            # v projection first: out [t, m]; memset ones then fill per-head
            # slices (attention consumes v from k-block 0 onward)
            wv_tiles = [load_w(wvT, dc) for dc in range(DC)]
            for tci in range(NTC):
                x_tiles = [load_x(dc, tci, 2) for dc in range(DC)]
                for tbi in range(TBC):
                    tb = tci * TBC + tbi
                    nc.gpsimd.memset(v_t[tb][:], 1.0)
                    ps = pps.tile([P, M], f32, tag="pps")
                    for dc in range(DC):
                        nc.tensor.matmul(
                            ps[:, :],
                            x_tiles[dc][:, tbi * P:(tbi + 1) * P],
                            wv_tiles[dc][:, :],
                            start=(dc == 0), stop=(dc == DC - 1))
                    for h in range(NH):
                        nc.vector.tensor_copy(
                            out=v_t[tb][:, h * HG:h * HG + HD],
                            in_=ps[:, h * HD:(h + 1) * HD])

            # fused k+q pass over shared x chunks: per tci, emit k then q
            # m-blocks so attention (which needs kT/qT m-block 0 and early
            # k-columns) can start while later chunks still project
            wk_tiles = [load_w(wkT, dc) for dc in range(DC)]
            wq_tiles = [load_w(wqT, dc) for dc in range(DC)]
            for tci in range(NTC):
                x_tiles = [load_x(dc, tci, 0) for dc in range(DC)]
                for w_tiles, bias_t, out_t in ((wk_tiles, bk_t, kT_t),
                                               (wq_tiles, bq_t, qT_t)):
                    for mb in range(MB):
                        ps = pps.tile([P, TC], f32, tag="pps")
                        for dc in range(DC):
                            nc.tensor.matmul(
                                ps[:, :],
                                w_tiles[dc][:, mb * P:(mb + 1) * P],
                                x_tiles[dc][:, :],
                                start=(dc == 0), stop=(dc == DC - 1))
                        nc.vector.tensor_scalar_add(
                            out_t[mb][:, tci * TC:(tci + 1) * TC],
                            ps[:, :], bias_t[:, mb:mb + 1])

        # ---- attention, one head pair at a time ----
        with tc.tile_pool(name="scps", bufs=2, space="PSUM") as scps, \
             tc.tile_pool(name="atps", bufs=2, space="PSUM") as atps:
            for j in range(NH // 2):          # head pair (2j, 2j+1)
                for qh in range(2):
                    q0 = qh * QH
                    at_ps = [atps.tile([P, QH], f32, tag="at",
                                       name=f"at{j}_{qh}_{s}") for s in range(2)]
                    for kb in range(TB):
                        sc_ps = [scps.tile([P, QH], f32, tag="sc",
                                           name=f"sc{j}_{qh}_{kb}_{s}")
                                 for s in range(2)]
                        # both heads' scores: 64-contraction at partition
                        # offsets 0/64 -> concurrent 64x128 PE row-tiles
                        for s in range(2):
                            r0 = s * HD
                            for qc in range(NQC):
                                nc.tensor.matmul(
                                    sc_ps[s][:, qc * QC:(qc + 1) * QC],
                                    kT_t[j][r0:r0 + HD, kb * P:(kb + 1) * P],
                                    qT_t[j][r0:r0 + HD,
                                            q0 + qc * QC:q0 + (qc + 1) * QC],
                                    start=True, stop=True)
                        exs = []
                        for s in range(2):
                            ex = big.tile([P, QH], bf16, tag="ex",
                                          name=f"ex{j}_{qh}_{kb}_{s}")
                            nc.scalar.activation(
                                out=ex[:, :], in_=sc_ps[s][:, :],
                                func=mybir.ActivationFunctionType.Exp)
                            exs.append(ex)
                        for s in range(2):
                            h = 2 * j + s
                            for qc in range(NQC):
                                nc.tensor.matmul(
                                    at_ps[s][:HG, qc * QC:(qc + 1) * QC],
                                    v_t[kb][:, h * HG:(h + 1) * HG],
                                    exs[s][:, qc * QC:(qc + 1) * QC],
                                    start=(kb == 0), stop=(kb == TB - 1))
                    # normalize: recip of sums row, broadcast via selector
                    # matmul, multiply into yT
                    for s in range(2):
                        r0 = s * HD
                        att = attp.tile([P, QH], f32r, tag="attT",
                                        name=f"att{j}_{qh}_{s}")
                        nc.vector.tensor_copy(out=att[:HG, :],
                                              in_=at_ps[s][:HG, :])
                        # 1/sums on the (otherwise idle) ScalarE:
                        # exp(-ln(sums)); spline error ~1e-6, far cheaper
                        # than DVE reciprocal (~6 cyc/elem/lane)
                        bc = atps.tile([P, QH], f32, tag="at",
                                       name=f"bc{j}_{qh}_{s}")
                        for qc in range(NQC):
                            nc.tensor.matmul(
                                bc[:HG, qc * QC:(qc + 1) * QC],
                                sel[:HG, :],
                                att[:HG, qc * QC:(qc + 1) * QC],
                                start=True, stop=True)
                        # evacuate the broadcast sums to SBUF fast (frees the
                        # PSUM slot for the next pair), then invert + multiply
                        # off the PE-critical path
                        bcs = attp.tile([P, QH], f32, tag="bcs",
                                        name=f"bcs{j}_{qh}_{s}")
                        nc.vector.tensor_copy(out=bcs[:HD, :], in_=bc[:HD, :])
                        nc.vector.reciprocal_approx_fast(out=bcs[:HD, :],
                                                         in_=bcs[:HD, :])
                        nc.vector.tensor_mul(
                            out=yT_t[j][r0:r0 + HD, q0:q0 + QH],
                            in0=att[:HD, :].bitcast(f32),
                            in1=bcs[:HD, :])

        # ---- finalize: residual (+bv) and store, in QH-wide chunks ----
        for mb in range(MB):
            for ci in range(2):
                xr = big.tile([P, QH], f32, tag="ex", name=f"xr{mb}_{ci}")
                nc.sync.dma_start(
                    xr[:], xresbT[mb * P:(mb + 1) * P, ci * QH:(ci + 1) * QH])
                nc.vector.tensor_add(
                    out=yT_t[mb][:, ci * QH:(ci + 1) * QH],
                    in0=yT_t[mb][:, ci * QH:(ci + 1) * QH], in1=xr[:])
                nc.sync.dma_start(
                    y[mb * P:(mb + 1) * P, ci * QH:(ci + 1) * QH],
                    yT_t[mb][:, ci * QH:(ci + 1) * QH])

    nc.compile()
    return nc


_compiled = None


def _get_compiled():
    global _compiled
    if _compiled is None:
        _compiled = build_program(**FULL_CFG)
    return _compiled


def shard_inputs(x, Wq, bq, Wk, bk, Wv, bv):
    """Build the 8 per-core input dicts (host-side prep)."""
    x = np.asarray(x, np.float32)
    in_maps = []
    for c in range(N_CORES):
        b, hh = divmod(c, 2)
        sl = slice(hh * 512, (hh + 1) * 512)
        in_maps.append({
            "xT": np.ascontiguousarray(x[b].T),
            "wqT": np.ascontiguousarray(np.asarray(Wq, np.float32)[sl, :].T),
            "wkT": np.ascontiguousarray(np.asarray(Wk, np.float32)[sl, :].T),
            "wvT": np.ascontiguousarray(np.asarray(Wv, np.float32)[sl, :].T),
            "bqT": np.ascontiguousarray(
                np.asarray(bq, np.float32)[sl].reshape(4, 128).T),
            "bkT": np.ascontiguousarray(
                np.asarray(bk, np.float32)[sl].reshape(4, 128).T),
            "xresbT": np.ascontiguousarray(
                (x[b][:, sl] + np.asarray(bv, np.float32)[sl][None, :]).T),
        })
    return in_maps


def run_sharded(in_maps, trace=False, **kw):
    from concourse.bass_utils import run_bass_kernel_spmd
    nc = _get_compiled()
    return run_bass_kernel_spmd(nc, in_maps, core_ids=list(range(N_CORES)),
                                trace=trace, **kw)


def kernel(x, Wq, bq, Wk, bk, Wv, bv):
    res = run_sharded(shard_inputs(x, Wq, bq, Wk, bk, Wv, bv))
    out = np.empty((B, N, D), np.float32)
    for c in range(N_CORES):
        b, hh = divmod(c, 2)
        out[b][:, hh * 512:(hh + 1) * 512] = res.results[c]["y"].T
    return out


# revision 35
# speedup vs baseline: 1.0720x; 1.0720x over previous
"""Trainium2 Bass kernel for nn_Attention_41420664603305.

Multi-head self-attention block (16 heads, d_model=1024, head_dim=64, no
score scaling) with residual:  y = x + MHA(x).

Sharding over 8 NeuronCores: core c handles batch b = c//2 and heads
h0 = (c%2)*8 .. h0+8  (tensor-parallel column split of Wq/Wk/Wv).  Each core
computes y[b, :, (c%2)*512 : (c%2)*512+512] transposed; host re-transposes
and concatenates.

On-chip formulation (per core; T=2048 tokens, M=512 out cols, 8 heads):
  - Host pre-transposes: xT [1024, 2048], w{q,k,v}T [1024, 512],
    xresbT [512, 2048] (= (x residual + bv)^T; bv folds into the residual
    because softmax rows sum to exactly 1).
  - qT, kT = wT.T @ xT  ([m, t] layout, bias added per-partition on DVE).
  - v = xT.T @ wvT  ([t, m] layout), stored per-head as 65-wide groups with a
    trailing ones column, so attn@V also produces the softmax denominator.
  - Attention runs over head PAIRS (2j, 2j+1): the two heads' score matmuls
    have 64-deep contraction at partition offsets 0 / 64, so they lower to
    the two 64x128 PE row-tiles and execute concurrently.
    Per (pair, q-half, k-block): scoresT[k, q] (f32r) -> exp on ScalarE
    (no max subtraction; |scores| < ~50 fits fp32/exp fine) -> attnV
    accumulates attT[65, q] over k-blocks (row 64 = sums).
  - Normalize without transposes: reciprocal in place on attT row 64, then
    a selector matmul (sel.T @ att, sel = e_64 outer ones) broadcasts the
    recip row across partitions in PSUM; one DVE multiply writes the
    normalized head output straight into yT [m, t].
  - yT += xresbT, DMA out (y delivered transposed [M, T]).

All big matmuls run as float32r (full-rate fp32 mode on the PE).
"""

import sys

if "/opt/trn_rl_repo" not in sys.path:
    sys.path.insert(0, "/opt/trn_rl_repo")

import numpy as np

B, N, D, HEADS = 4, 2048, 1024, 16
HD = 64
N_CORES = 8

FULL_CFG = dict(T=N, DIN=D, M=D // 2, NH=HEADS // 2)


def build_program(T, DIN, M, NH, n_devices=N_CORES, enable_asserts=False):
    """Emit the Tile program; same SPMD program runs on every core."""
    from contextlib import ExitStack

    import concourse.bass as bass  # noqa: F401
    import concourse.tile as tile
    from concourse import bacc, mybir
    from concourse.masks import make_identity  # noqa: F401

    P = 128
    HG = HD + 1                    # per-head group width in v (64 vals + one)
    f32 = mybir.dt.float32
    f32r = mybir.dt.float32r
    bf16 = mybir.dt.bfloat16
    DC = DIN // P                  # contraction chunks
    MB = M // P                    # m-blocks (2 heads each)
    TB = T // P                    # token blocks
    TC = min(512, T)               # moving chunk for projections
    NTC = T // TC
    QH = T // 2                    # q processed in halves
    QC = min(512, QH)              # scores/attnV moving chunk
    NQC = QH // QC

    nc = bacc.Bacc("TRN2", target_bir_lowering=False, debug=False,
                   enable_asserts=enable_asserts, num_devices=n_devices)

    xT = nc.dram_tensor("xT", [DIN, T], f32r, kind="ExternalInput").ap()
    wqT = nc.dram_tensor("wqT", [DIN, M], f32r, kind="ExternalInput").ap()
    wkT = nc.dram_tensor("wkT", [DIN, M], f32r, kind="ExternalInput").ap()
    wvT = nc.dram_tensor("wvT", [DIN, M], f32r, kind="ExternalInput").ap()
    bqT = nc.dram_tensor("bqT", [P, MB], f32, kind="ExternalInput").ap()
    bkT = nc.dram_tensor("bkT", [P, MB], f32, kind="ExternalInput").ap()
    xresbT = nc.dram_tensor("xresbT", [M, T], f32, kind="ExternalInput").ap()
    y = nc.dram_tensor("y", [M, T], f32, kind="ExternalOutput").ap()

    with tile.TileContext(nc) as tc, ExitStack() as ctx:
        singles = ctx.enter_context(tc.tile_pool(name="singles", bufs=1))
        xp = ctx.enter_context(tc.tile_pool(name="xp", bufs=min(10, 2 * DC)))
        big = ctx.enter_context(tc.tile_pool(name="big", bufs=6))
        attp = ctx.enter_context(tc.tile_pool(name="attp", bufs=2))
        qk = ctx.enter_context(tc.tile_pool(name="qk", bufs=MB))
        vp = ctx.enter_context(tc.tile_pool(name="vp", bufs=TB))
        wp = ctx.enter_context(tc.tile_pool(name="wp", bufs=min(16, 3 * DC)))
        yp = ctx.enter_context(tc.tile_pool(name="yp", bufs=MB))

        # constants: selector (row HD = ones, else 0) and per-partition biases
        sel = singles.tile([P, HG], f32r, tag="sel")
        nc.vector.memset(sel[:].bitcast(f32), 0.0)
        nc.vector.memset(sel[HD:HG, :].bitcast(f32), 1.0)
        bq_t = singles.tile([P, MB], f32, tag="bq")
        nc.sync.dma_start(bq_t[:], bqT[:, :])
        bk_t = singles.tile([P, MB], f32, tag="bk")
        nc.sync.dma_start(bk_t[:], bkT[:, :])

        def load_x(dc, tci, tag):
            t = xp.tile([P, TC], f32r, tag="x", name=f"x{tag}_{dc}_{tci}")
            eng = nc.sync if dc % 2 == 0 else nc.gpsimd
            eng.dma_start(t[:], xT[dc * P:(dc + 1) * P,
                                   tci * TC:(tci + 1) * TC])
            return t

        def load_w(w_ap, dc):
            t = wp.tile([P, M], f32r, tag="w")
            eng = nc.gpsimd if dc % 2 == 0 else nc.sync
            eng.dma_start(t[:], w_ap[dc * P:(dc + 1) * P, :])
            return t

        qT_t = [qk.tile([P, T], f32r, tag="qT", name=f"qT{i}") for i in range(MB)]
        kT_t = [qk.tile([P, T], f32r, tag="kT", name=f"kT{i}") for i in range(MB)]
        v_t = [vp.tile([P, NH * HG], bf16, tag="v", name=f"v{i}") for i in range(TB)]
        yT_t = [yp.tile([P, T], f32, tag="y", name=f"yT{i}") for i in range(MB)]

        # ---- projections (x streamed per pass in TC-column chunks) ----
        TBC = TC // P  # token blocks per x chunk column group
        psum = ctx.enter_context(tc.tile_pool(name="psum", bufs=2, space="PSUM"))
        if True:
            # v projection first: out [t, m]; memset ones then fill per-head
            # slices (attention consumes v from k-block 0 onward)
            wv_tiles = [load_w(wvT, dc) for dc in range(DC)]
            for tci in range(NTC):
                x_tiles = [load_x(dc, tci, 2) for dc in range(DC)]
                for tbi in range(TBC):
                    tb = tci * TBC + tbi
                    nc.gpsimd.memset(v_t[tb][:], 1.0)
                    ps = psum.tile([P, QH], f32, tag="sc", name=f"pv{tb}")[:, :M]
                    for dc in range(DC):
                        nc.tensor.matmul(
                            ps[:, :],
                            x_tiles[dc][:, tbi * P:(tbi + 1) * P],
                            wv_tiles[dc][:, :],
                            start=(dc == 0), stop=(dc == DC - 1))
                    for h in range(NH):
                        nc.vector.tensor_copy(
                            out=v_t[tb][:, h * HG:h * HG + HD],
                            in_=ps[:, h * HD:(h + 1) * HD])

            # fused k+q pass over shared x chunks: per tci, emit k then q
            # m-blocks so attention (which needs kT/qT m-block 0 and early
            # k-columns) can start while later chunks still project
            wk_tiles = [load_w(wkT, dc) for dc in range(DC)]
            wq_tiles = [load_w(wqT, dc) for dc in range(DC)]
            for tci in range(NTC):
                x_tiles = [load_x(dc, tci, 0) for dc in range(DC)]
                for w_tiles, bias_t, out_t in ((wk_tiles, bk_t, kT_t),
                                               (wq_tiles, bq_t, qT_t)):
                    for mb in range(MB):
                        ps = psum.tile([P, QH], f32, tag="sc", name=f"pp{tci}_{mb}")[:, :TC]
                        for dc in range(DC):
                            nc.tensor.matmul(
                                ps[:, :],
                                w_tiles[dc][:, mb * P:(mb + 1) * P],
                                x_tiles[dc][:, :],
                                start=(dc == 0), stop=(dc == DC - 1))
                        nc.vector.tensor_scalar_add(
                            out_t[mb][:, tci * TC:(tci + 1) * TC],
                            ps[:, :], bias_t[:, mb:mb + 1])

        # ---- attention, one head pair at a time ----
        if True:
            for j in range(NH // 2):          # head pair (2j, 2j+1)
                for qh in range(2):
                    q0 = qh * QH
                    at_ps = [psum.tile([P, QH], f32, tag="at",
                                       name=f"at{j}_{qh}_{s}") for s in range(2)]
                    for kb in range(TB):
                        sc_ps = [psum.tile([P, QH], f32, tag="sc",
                                           name=f"sc{j}_{qh}_{kb}_{s}")
                                 for s in range(2)]
                        # both heads' scores: 64-contraction at partition
                        # offsets 0/64 -> concurrent 64x128 PE row-tiles
                        for s in range(2):
                            r0 = s * HD
                            for qc in range(NQC):
                                nc.tensor.matmul(
                                    sc_ps[s][:, qc * QC:(qc + 1) * QC],
                                    kT_t[j][r0:r0 + HD, kb * P:(kb + 1) * P],
                                    qT_t[j][r0:r0 + HD,
                                            q0 + qc * QC:q0 + (qc + 1) * QC],
                                    start=True, stop=True)
                        exs = []
                        for s in range(2):
                            ex = big.tile([P, QH], bf16, tag="ex",
                                          name=f"ex{j}_{qh}_{kb}_{s}")
                            nc.scalar.activation(
                                out=ex[:, :], in_=sc_ps[s][:, :],
                                func=mybir.ActivationFunctionType.Exp)
                            exs.append(ex)
                        for s in range(2):
                            h = 2 * j + s
                            for qc in range(NQC):
                                nc.tensor.matmul(
                                    at_ps[s][:HG, qc * QC:(qc + 1) * QC],
                                    v_t[kb][:, h * HG:(h + 1) * HG],
                                    exs[s][:, qc * QC:(qc + 1) * QC],
                                    start=(kb == 0), stop=(kb == TB - 1))
                    # normalize: recip of sums row, broadcast via selector
                    # matmul, multiply into yT
                    for s in range(2):
                        r0 = s * HD
                        att = attp.tile([P, QH], f32r, tag="attT",
                                        name=f"att{j}_{qh}_{s}")
                        nc.vector.tensor_copy(out=att[:HG, :],
                                              in_=at_ps[s][:HG, :])
                        # 1/sums on the (otherwise idle) ScalarE:
                        # exp(-ln(sums)); spline error ~1e-6, far cheaper
                        # than DVE reciprocal (~6 cyc/elem/lane)
                        bc = psum.tile([P, QH], f32, tag="at",
                                       name=f"bc{j}_{qh}_{s}")
                        for qc in range(NQC):
                            nc.tensor.matmul(
                                bc[:HG, qc * QC:(qc + 1) * QC],
                                sel[:HG, :],
                                att[:HG, qc * QC:(qc + 1) * QC],
                                start=True, stop=True)
                        # evacuate the broadcast sums to SBUF fast (frees the
                        # PSUM slot for the next pair), then invert + multiply
                        # off the PE-critical path
                        bcs = attp.tile([P, QH], f32, tag="bcs",
                                        name=f"bcs{j}_{qh}_{s}")
                        nc.vector.tensor_copy(out=bcs[:HD, :], in_=bc[:HD, :])
                        nc.vector.reciprocal_approx_fast(out=bcs[:HD, :],
                                                         in_=bcs[:HD, :])
                        nc.vector.tensor_mul(
                            out=yT_t[j][r0:r0 + HD, q0:q0 + QH],
                            in0=att[:HD, :].bitcast(f32),
                            in1=bcs[:HD, :])

        # ---- finalize: residual (+bv) and store, in QH-wide chunks ----
        for mb in range(MB):
            for ci in range(2):
                xr = big.tile([P, QH], f32, tag="ex", name=f"xr{mb}_{ci}")
                nc.sync.dma_start(
                    xr[:], xresbT[mb * P:(mb + 1) * P, ci * QH:(ci + 1) * QH])
                nc.vector.tensor_add(
                    out=yT_t[mb][:, ci * QH:(ci + 1) * QH],
                    in0=yT_t[mb][:, ci * QH:(ci + 1) * QH], in1=xr[:])
                nc.sync.dma_start(
                    y[mb * P:(mb + 1) * P, ci * QH:(ci + 1) * QH],
                    yT_t[mb][:, ci * QH:(ci + 1) * QH])

    nc.compile()
    return nc


_compiled = None


def _get_compiled():
    global _compiled
    if _compiled is None:
        _compiled = build_program(**FULL_CFG)
    return _compiled


def shard_inputs(x, Wq, bq, Wk, bk, Wv, bv):
    """Build the 8 per-core input dicts (host-side prep)."""
    x = np.asarray(x, np.float32)
    in_maps = []
    for c in range(N_CORES):
        b, hh = divmod(c, 2)
        sl = slice(hh * 512, (hh + 1) * 512)
        in_maps.append({
            "xT": np.ascontiguousarray(x[b].T),
            "wqT": np.ascontiguousarray(np.asarray(Wq, np.float32)[sl, :].T),
            "wkT": np.ascontiguousarray(np.asarray(Wk, np.float32)[sl, :].T),
            "wvT": np.ascontiguousarray(np.asarray(Wv, np.float32)[sl, :].T),
            "bqT": np.ascontiguousarray(
                np.asarray(bq, np.float32)[sl].reshape(4, 128).T),
            "bkT": np.ascontiguousarray(
                np.asarray(bk, np.float32)[sl].reshape(4, 128).T),
            "xresbT": np.ascontiguousarray(
                (x[b][:, sl] + np.asarray(bv, np.float32)[sl][None, :]).T),
        })
    return in_maps


def run_sharded(in_maps, trace=False, **kw):
    from concourse.bass_utils import run_bass_kernel_spmd
    nc = _get_compiled()
    return run_bass_kernel_spmd(nc, in_maps, core_ids=list(range(N_CORES)),
                                trace=trace, **kw)


def kernel(x, Wq, bq, Wk, bk, Wv, bv):
    res = run_sharded(shard_inputs(x, Wq, bq, Wk, bk, Wv, bv))
    out = np.empty((B, N, D), np.float32)
    for c in range(N_CORES):
        b, hh = divmod(c, 2)
        out[b][:, hh * 512:(hh + 1) * 512] = res.results[c]["y"].T
    return out
